# revision 6
# baseline (speedup 1.0000x reference)
"""Trainium2 Bass kernel for nn_AutoEncoder (gnn_message_passing).

Strategy: pure data-parallel over the batch (cells) axis across 8 NeuronCores.
Each core handles 128 of the 1024 cells end-to-end; the sparse Laplacian is
densified to bf16 on host and streamed from HBM as the moving operand of the
two Chebyshev SpMMs. All matmuls run bf16 x bf16 -> f32 PSUM.

Self-contained: hardcodes shapes; no sibling imports.
"""
import sys

sys.path.insert(0, "/opt/trn_rl_repo")

import numpy as np
import ml_dtypes

import concourse.bass as bass
import concourse.mybir as mybir
from concourse import tile, bacc
from concourse.bass_utils import run_bass_kernel_spmd

BF16 = mybir.dt.bfloat16
F32 = mybir.dt.float32
AF = mybir.ActivationFunctionType
ALU = mybir.AluOpType

V = 8192
B = 1024
IN = 20000
IN_PAD = 20480          # 160 * 128 = 40 * 512
OUT = 64
NCORES = 8
BS = B // NCORES        # 128 batch rows per core
NKV = V // 128          # 64 contraction chunks for the SpMM
NK0 = IN_PAD // 128     # 160 contraction chunks for enc0
NT = IN_PAD // 512      # 40 output tiles for sig / recon

LAST_EXEC_NS = None

bf16 = ml_dtypes.bfloat16


def _chunk128(wt):
    """[K, N] (K % 128 == 0) -> [128, (K//128)*N] with cw[p, k*N+j] = wt[k*128+p, j]."""
    K, N = wt.shape
    return np.ascontiguousarray(
        wt.reshape(K // 128, 128, N).transpose(1, 0, 2).reshape(128, -1)
    )


def _build(nc, cl1_w, cl1_b):
    """Builds the per-core graph. cl1_w [3,3], cl1_b [3] are baked as immediates."""
    dp = nc.declare_dram_parameter

    x0c_d = dp("x0c", [128, NKV * 128], BF16, isOutput=False)     # x_gcn.T lhsT chunks
    x0t_d = dp("x0t", [128, V], BF16, isOutput=False)             # x_gcn shard [b, v]
    xnc_d = dp("xnc", [128, NK0 * 128], BF16, isOutput=False)     # x_nn.T lhsT chunks
    lt_d = dp("lt", [V, V], BF16, isOutput=False)                 # dense L^T
    eye_d = dp("eye", [128, 128], BF16, isOutput=False)

    e0w_d = dp("e0w", [128, NK0 * 512], BF16, isOutput=False)
    e1w_d = dp("e1w", [128, 4 * 256], BF16, isOutput=False)
    e2w_d = dp("e2w", [128, 2 * 128], BF16, isOutput=False)
    e3w_d = dp("e3w", [128, 64], BF16, isOutput=False)
    e4w_d = dp("e4w", [64, 32], BF16, isOutput=False)
    eb_d = [dp(f"e{i}b", [1, n], BF16, isOutput=False)
            for i, n in enumerate([512, 256, 128, 64, 32])]

    f1w_d = dp("f1w", [128, 24 * 512], BF16, isOutput=False)
    f2w_d = dp("f2w", [128, 4 * 256], BF16, isOutput=False)
    f3w_d = dp("f3w", [128, 2 * 32], BF16, isOutput=False)
    f1b_d = dp("f1b", [1, 512], BF16, isOutput=False)
    f2b_d = dp("f2b", [1, 256], BF16, isOutput=False)
    f3b_d = dp("f3b", [1, 32], BF16, isOutput=False)

    d0_d = dp("d0", [64, 64], BF16, isOutput=False)               # dec0_w
    d1t_d = dp("d1t", [64, 128], BF16, isOutput=False)            # dec1_w.T
    d2t_d = dp("d2t", [128, 2 * 128], BF16, isOutput=False)       # dec2_w.T chunked
    d3t_d = dp("d3t", [128, 2 * 512], BF16, isOutput=False)       # dec3_w.T chunked
    d4b_d = dp("d4b", [128, NT * 4 * 512], BF16, isOutput=False)  # dec4_w.T blocked
    sw_d = dp("sw", [32, 64], BF16, isOutput=False)               # sum_w.T
    sb_d = dp("sb", [1, 64], BF16, isOutput=False)

    recon_d = dp("out_recon", [128, IN], F32, isOutput=True)
    z_d = dp("out_z", [128, OUT], F32, isOutput=True)
    sig_d = dp("out_sig", [64, IN], F32, isOutput=True)

    with tile.TileContext(nc) as tc:
        with (
            tc.tile_pool(name="const", bufs=1) as cp,
            tc.tile_pool(name="res", bufs=1) as rp,
            tc.tile_pool(name="lstream", bufs=2) as lsp,
            tc.tile_pool(name="wstream", bufs=2) as wsp,
            tc.tile_pool(name="work", bufs=2) as wp,
            tc.tile_pool(name="psA", bufs=4, space="PSUM") as psA,
            tc.tile_pool(name="psB", bufs=4, space="PSUM") as psB,
        ):
            # ---- constants / resident tensors ----
            eye = cp.tile([128, 128], BF16, tag="eye")
            nc.sync.dma_start(eye[:], eye_d[:])
            ones = cp.tile([1, 128], BF16, tag="ones")
            nc.vector.memset(ones[:], 1.0)
            cbs = []
            for f in range(3):
                cbt = cp.tile([128, 1], F32, tag=f"cb{f}", name=f"cb{f}")
                nc.vector.memset(cbt[:], float(cl1_b[f]))
                cbs.append(cbt)

            x0c = rp.tile([128, NKV * 128], BF16, tag="x0c")
            nc.sync.dma_start(x0c[:], x0c_d[:])
            x0t = rp.tile([128, V], BF16, tag="x0t")
            nc.sync.dma_start(x0t[:], x0t_d[:])
            x1t = rp.tile([128, V], BF16, tag="x1t")
            x1c = rp.tile([128, NKV * 128], BF16, tag="x1c")
            xp = rp.tile([128, 3 * 1024], BF16, tag="xp")   # pooled planes, f-major

            e1w = cp.tile([128, 4 * 256], BF16, tag="e1w")
            nc.sync.dma_start(e1w[:], e1w_d[:])
            e2w = cp.tile([128, 2 * 128], BF16, tag="e2w")
            nc.sync.dma_start(e2w[:], e2w_d[:])
            e3w = cp.tile([128, 64], BF16, tag="e3w")
            nc.sync.dma_start(e3w[:], e3w_d[:])
            e4w = cp.tile([64, 32], BF16, tag="e4w")
            nc.sync.dma_start(e4w[:], e4w_d[:])
            ebs = []
            for i, n in enumerate([512, 256, 128, 64, 32]):
                t = cp.tile([1, n], BF16, tag=f"eb{i}")
                nc.sync.dma_start(t[:], eb_d[i][:])
                ebs.append(t)
            f1w = cp.tile([128, 24 * 512], BF16, tag="f1w")
            nc.sync.dma_start(f1w[:], f1w_d[:])
            f2w = cp.tile([128, 4 * 256], BF16, tag="f2w")
            nc.sync.dma_start(f2w[:], f2w_d[:])
            f3w = cp.tile([128, 2 * 32], BF16, tag="f3w")
            nc.sync.dma_start(f3w[:], f3w_d[:])
            f1b = cp.tile([1, 512], BF16, tag="f1b")
            nc.sync.dma_start(f1b[:], f1b_d[:])
            f2b = cp.tile([1, 256], BF16, tag="f2b")
            nc.sync.dma_start(f2b[:], f2b_d[:])
            f3b = cp.tile([1, 32], BF16, tag="f3b")
            nc.sync.dma_start(f3b[:], f3b_d[:])
            d0 = cp.tile([64, 64], BF16, tag="d0")
            nc.sync.dma_start(d0[:], d0_d[:])
            d1t = cp.tile([64, 128], BF16, tag="d1t")
            nc.sync.dma_start(d1t[:], d1t_d[:])
            d2t = cp.tile([128, 2 * 128], BF16, tag="d2t")
            nc.sync.dma_start(d2t[:], d2t_d[:])
            d3t = cp.tile([128, 2 * 512], BF16, tag="d3t")
            nc.sync.dma_start(d3t[:], d3t_d[:])
            sw = cp.tile([32, 64], BF16, tag="sw")
            nc.sync.dma_start(sw[:], sw_d[:])
            sb = cp.tile([1, 64], BF16, tag="sb")
            nc.sync.dma_start(sb[:], sb_d[:])

            def transpose128(dst_ap, src_ap):
                """PE transpose of a [128,128] bf16 tile via identity."""
                pt = psB.tile([128, 128], BF16, tag="mm")
                nc.tensor.transpose(pt[:], src_ap, eye[:])
                nc.vector.tensor_copy(dst_ap, pt[:])

            # =========== GCN branch: two SpMM passes over dense L^T ===========
            def spmm(lhs_chunks, out_bf, out_chunks, is_second):
                for q in range(4):
                    blocks = [psA.tile([128, 512], F32, tag="acc", name=f"psblk{q}_{i}") for i in range(4)]
                    for k in range(NKV):
                        lt_sb = lsp.tile([128, 2048], BF16, tag="lt", name=f"lt{q}_{k}")
                        nc.sync.dma_start(
                            lt_sb[:],
                            lt_d[k * 128:(k + 1) * 128,
                                 q * 2048:(q + 1) * 2048],
                        )
                        for vb in range(4):
                            nc.tensor.matmul(
                                blocks[vb][:],
                                lhs_chunks[:, k * 128:(k + 1) * 128],
                                lt_sb[:, vb * 512:(vb + 1) * 512],
                                start=(k == 0),
                                stop=(k == NKV - 1),
                                skip_group_check=True,
                            )
                    for vb in range(4):
                        blk = q * 4 + vb
                        col = blk * 512
                        ps = blocks[vb]
                        if not is_second:
                            # x1 = L x0 : stash bf16 copy + transposed chunks
                            nc.scalar.activation(
                                out_bf[:, col:col + 512], ps[:], AF.Copy)
                            for j in range(4):
                                transpose128(
                                    out_chunks[:, col + j * 128: col + (j + 1) * 128],
                                    out_bf[:, col + j * 128: col + (j + 1) * 128],
                                )
                        else:
                            # x2 = 2 (L x1) - x0 ; then Chebyshev combine + relu
                            # + window-8 max pool, all on this [128,512] block.
                            for f in range(3):
                                w0 = float(cl1_w[f, 0] - cl1_w[f, 2])
                                w1 = float(cl1_w[f, 1])
                                w2 = float(2.0 * cl1_w[f, 2])
                                ta = wp.tile([128, 512], BF16, tag="cmb_a")
                                nc.vector.tensor_scalar_mul(
                                    ta[:], x0t[:, col:col + 512], w0)
                                tb = wp.tile([128, 512], BF16, tag="cmb_b")
                                nc.vector.scalar_tensor_tensor(
                                    tb[:], x1t[:, col:col + 512], w1, ta[:],
                                    op0=ALU.mult, op1=ALU.add)
                                tg = wp.tile([128, 512], BF16, tag="cmb_g")
                                nc.vector.scalar_tensor_tensor(
                                    tg[:], ps[:], w2, tb[:],
                                    op0=ALU.mult, op1=ALU.add)
                                tr = wp.tile([128, 512], BF16, tag="cmb_r")
                                nc.scalar.activation(tr[:], tg[:], AF.Relu, bias=cbs[f][:])
                                nc.vector.tensor_reduce(
                                    xp[:, f * 1024 + blk * 64:
                                       f * 1024 + (blk + 1) * 64],
                                    tr.rearrange("p (a w) -> p a w", w=8),
                                    axis=mybir.AxisListType.X,
                                    op=ALU.max,
                                )

            spmm(x0c, x1t, x1c, is_second=False)
            spmm(x1c, None, None, is_second=True)

            # =========== GCN fc head ===========
            featc = rp.tile([128, 24 * 128], BF16, tag="featc")
            for c in range(24):
                transpose128(featc[:, c * 128:(c + 1) * 128],
                             xp[:, c * 128:(c + 1) * 128])

            def dense(lhs_chunks, nk, wsb, n_out, bias, psum_tag="mm"):
                ps = psB.tile([128, n_out], F32, tag="mm")
                for k in range(nk):
                    nc.tensor.matmul(
                        ps[:], lhs_chunks[:, k * 128:(k + 1) * 128],
                        wsb[:, k * n_out:(k + 1) * n_out],
                        start=(k == 0), stop=False, skip_group_check=True)
                nc.tensor.matmul(ps[:], ones[:], bias[:],
                                 start=False, stop=True, skip_group_check=True)
                return ps

            def relu_t(ps, n):
                t = wp.tile([128, n], BF16, tag="relu")
                nc.scalar.activation(t[:], ps[:], AF.Relu)
                return t

            def celu_t(ps, n):
                pos = wp.tile([128, n], F32, tag="celu_p")
                nc.scalar.activation(pos[:], ps[:], AF.Relu)
                m = wp.tile([128, n], F32, tag="celu_m")
                nc.vector.tensor_scalar_min(m[:], ps[:], 0.0)
                e = wp.tile([128, n], F32, tag="celu_e")
                nc.scalar.activation(e[:], m[:], AF.Exp)
                h = wp.tile([128, n], BF16, tag="celu_h")
                nc.vector.scalar_tensor_tensor(
                    h[:], e[:], -1.0, pos[:], op0=ALU.add, op1=ALU.add)
                return h

            def tchunks(src, n):
                """bf16 [128, n] -> transposed chunks [128, (n//128)*128]."""
                dst = wp.tile([128, n], BF16, tag=f"tc{n}")
                for j in range(n // 128):
                    transpose128(dst[:, j * 128:(j + 1) * 128],
                                 src[:, j * 128:(j + 1) * 128])
                return dst

            g1 = relu_t(dense(featc, 24, f1w, 512, f1b), 512)
            g1c = tchunks(g1, 512)
            g2 = relu_t(dense(g1c, 4, f2w, 256, f2b), 256)
            g2c = tchunks(g2, 256)
            ps_gf = dense(g2c, 2, f3w, 32, f3b)  # x_gcn_feat [128,32]
            gf = wp.tile([128, 32], F32, tag="gf")
            nc.vector.tensor_copy(gf[:], ps_gf[:])

            # =========== encoder branch ===========
            ps_h0 = psB.tile([128, 512], F32, tag="mm")
            for g in range(NK0 // 8):
                xg_sb = wsp.tile([128, 8 * 128], BF16, tag="xn_g")
                nc.sync.dma_start(xg_sb[:], xnc_d[:, g * 1024:(g + 1) * 1024])
                w_sb = wsp.tile([128, 8 * 512], BF16, tag="e0w_g")
                nc.sync.dma_start(w_sb[:], e0w_d[:, g * 4096:(g + 1) * 4096])
                for kk in range(8):
                    k = g * 8 + kk
                    nc.tensor.matmul(
                        ps_h0[:], xg_sb[:, kk * 128:(kk + 1) * 128],
                        w_sb[:, kk * 512:(kk + 1) * 512],
                        start=(k == 0), stop=False, skip_group_check=True)
            nc.tensor.matmul(ps_h0[:], ones[:], ebs[0][:],
                             start=False, stop=True, skip_group_check=True)
            h0 = celu_t(ps_h0, 512)
            h0c = tchunks(h0, 512)
            h1 = celu_t(dense(h0c, 4, e1w, 256, ebs[1]), 256)
            h1c = tchunks(h1, 256)
            h2 = celu_t(dense(h1c, 2, e2w, 128, ebs[2]), 128)
            h2c = tchunks(h2, 128)
            h3 = celu_t(dense(h2c, 1, e3w, 64, ebs[3]), 64)
            h3c = wp.tile([64, 128], BF16, tag="h3c")
            pt3 = psB.tile([64, 128], BF16, tag="mm")
            nc.tensor.transpose(pt3[:], h3[:, 0:64], eye[:])
            nc.vector.tensor_copy(h3c[:], pt3[:])
            ps_nf = psB.tile([128, 32], F32, tag="mm")
            nc.tensor.matmul(ps_nf[:], h3c[:], e4w[:],
                             start=True, stop=False, skip_group_check=True)
            nc.tensor.matmul(ps_nf[:], ones[:], ebs[4][:],
                             start=False, stop=True, skip_group_check=True)

            # =========== fuse + softmax ===========
            xs = wp.tile([128, 32], BF16, tag="xs")
            nc.vector.scalar_tensor_tensor(
                xs[:], ps_nf[:], 1.0, gf[:], op0=ALU.mult, op1=ALU.add)
            nc.vector.tensor_scalar_mul(xs[:], xs[:], 0.5)
            xst = wp.tile([32, 128], BF16, tag="xst")
            ptx = psB.tile([32, 128], BF16, tag="mm")
            nc.tensor.transpose(ptx[:], xs[:, 0:32], eye[:])
            nc.vector.tensor_copy(xst[:], ptx[:])
            ps_lg = psB.tile([128, 64], F32, tag="mm")
            nc.tensor.matmul(ps_lg[:], xst[:], sw[:],
                             start=True, stop=False, skip_group_check=True)
            nc.tensor.matmul(ps_lg[:], ones[:], sb[:],
                             start=False, stop=True, skip_group_check=True)
            mx = wp.tile([128, 1], F32, tag="mx")
            nc.vector.tensor_reduce(mx[:], ps_lg[:],
                                    axis=mybir.AxisListType.X, op=ALU.max)
            nmx = wp.tile([128, 1], F32, tag="nmx")
            nc.vector.tensor_scalar_mul(nmx[:], mx[:], -1.0)
            ez = wp.tile([128, 64], F32, tag="ez")
            nc.scalar.activation(ez[:], ps_lg[:], AF.Exp, bias=nmx[:])
            ssum = wp.tile([128, 1], F32, tag="ssum")
            nc.vector.tensor_reduce(ssum[:], ez[:],
                                    axis=mybir.AxisListType.X, op=ALU.add)
            rs = wp.tile([128, 1], F32, tag="rs")
            nc.vector.reciprocal(rs[:], ssum[:])
            zf = wp.tile([128, 64], F32, tag="zf")
            nc.vector.tensor_scalar_mul(zf[:], ez[:], rs[:])
            nc.sync.dma_start(z_d[:], zf[:])
            zb = wp.tile([128, 64], BF16, tag="zb")
            nc.vector.tensor_copy(zb[:], zf[:])
            zt = wp.tile([64, 128], BF16, tag="zt")
            ptz = psB.tile([64, 128], BF16, tag="mm")
            nc.tensor.transpose(ptz[:], zb[:, 0:64], eye[:])
            nc.vector.tensor_copy(zt[:], ptz[:])

            # =========== sigmatrix chain ===========
            ps_m1 = psB.tile([128, 64], F32, tag="mm")
            nc.tensor.matmul(ps_m1[:], d1t[:], d0[:], start=True, stop=True,
                             skip_group_check=True)
            m1 = wp.tile([128, 64], BF16, tag="m1")
            nc.vector.tensor_copy(m1[:], ps_m1[:])
            m2 = wp.tile([128, 2 * 64], BF16, tag="m2")
            for c in range(2):
                psm = psB.tile([128, 64], F32, tag="mm")
                nc.tensor.matmul(psm[:], d2t[:, c * 128:(c + 1) * 128], m1[:],
                                 start=True, stop=True, skip_group_check=True)
                nc.vector.tensor_copy(m2[:, c * 64:(c + 1) * 64], psm[:])
            m3 = wp.tile([128, 4 * 64], BF16, tag="m3")
            for mchunk in range(4):
                psm = psB.tile([128, 64], F32, tag="mm")
                for k in range(2):
                    nc.tensor.matmul(
                        psm[:],
                        d3t[:, k * 512 + mchunk * 128: k * 512 + (mchunk + 1) * 128],
                        m2[:, k * 64:(k + 1) * 64],
                        start=(k == 0), stop=(k == 1), skip_group_check=True)
                nc.vector.tensor_copy(m3[:, mchunk * 64:(mchunk + 1) * 64], psm[:])

            # =========== sig tiles + reconstruction ===========
            for t in range(NT):
                d4_sb = wsp.tile([128, 4 * 512], BF16, tag="d4t")
                nc.sync.dma_start(d4_sb[:], d4b_d[:, t * 2048:(t + 1) * 2048])
                ps_s = psB.tile([64, 512], F32, tag="mm")
                for k in range(4):
                    nc.tensor.matmul(
                        ps_s[:], m3[:, k * 64:(k + 1) * 64],
                        d4_sb[:, k * 512:(k + 1) * 512],
                        start=(k == 0), stop=(k == 3), skip_group_check=True)
                sclip = wp.tile([64, 512], F32, tag="sclip")
                nc.vector.tensor_scalar(sclip[:], ps_s[:], 0.0, 1.0,
                                        op0=ALU.max, op1=ALU.min)
                sigb = wp.tile([64, 512], BF16, tag="sigb")
                nc.vector.tensor_copy(sigb[:], sclip[:])
                w = min(512, IN - t * 512)
                if w > 0:
                    nc.sync.dma_start(sig_d[:, t * 512:t * 512 + w], sclip[:, 0:w])
                ps_r = psB.tile([128, 512], F32, tag="mm")
                nc.tensor.matmul(ps_r[:], zt[:], sigb[:],
                                 start=True, stop=True, skip_group_check=True)
                rtile = wp.tile([128, 512], F32, tag="rtile")
                nc.vector.tensor_copy(rtile[:], ps_r[:])
                if w > 0:
                    nc.sync.dma_start(recon_d[:, t * 512:t * 512 + w], rtile[:, 0:w])

    nc.compile()
    return nc


def _time_exec(nc, in_maps, iters=6):
    """Median wall time of the sharded PJRT executable with device-resident
    inputs (approximates NEFF exec time; upload excluded)."""
    import time
    import jax
    from jax.sharding import Mesh, PartitionSpec, NamedSharding
    from jax.experimental.shard_map import shard_map
    from concourse import bass2jax, mybir as mb

    partition_name = nc.partition_id_tensor.name if nc.partition_id_tensor else None
    in_names, out_names, out_avals, zero_outs = [], [], [], []
    for alloc in nc.m.functions[0].allocations:
        if not isinstance(alloc, mb.MemoryLocationSet):
            continue
        name = alloc.memorylocations[0].name
        if alloc.kind == "ExternalInput":
            if name != partition_name:
                in_names.append(name)
        elif alloc.kind == "ExternalOutput":
            shape = tuple(alloc.tensor_shape)
            dtype = mb.dt.np(alloc.dtype)
            out_names.append(name)
            out_avals.append(jax.core.ShapedArray(shape, dtype))
            zero_outs.append(np.zeros(shape, dtype))
    n_params = len(in_names)
    all_names = in_names + out_names
    if partition_name is not None:
        all_names = all_names + [partition_name]

    def _body(*args):
        operands = list(args)
        if partition_name is not None:
            operands.append(bass2jax.partition_id_tensor())
        outs = bass2jax._bass_exec_p.bind(
            *operands,
            out_avals=tuple(out_avals),
            in_names=tuple(all_names),
            out_names=tuple(out_names),
            lowering_input_output_aliases=(),
            sim_require_finite=True,
            sim_require_nnan=True,
            nc=nc,
        )
        return tuple(outs)

    devices = jax.devices()[:NCORES]
    mesh = Mesh(np.asarray(devices), ("core",))
    spec = PartitionSpec("core")
    nshard = NamedSharding(mesh, spec)
    fn = jax.jit(
        shard_map(_body, mesh=mesh,
                  in_specs=(spec,) * (n_params + len(out_names)),
                  out_specs=(spec,) * len(out_names), check_rep=False),
        keep_unused=True,
    )
    args = []
    for i, name in enumerate(in_names):
        cat = np.concatenate([m[name] for m in in_maps], axis=0)
        args.append(jax.device_put(cat, nshard))
    for z in zero_outs:
        cat = np.concatenate([z] * NCORES, axis=0)
        args.append(jax.device_put(cat, nshard))
    times = []
    for it in range(iters):
        t0 = time.perf_counter()
        out = fn(*args)
        jax.block_until_ready(out)
        times.append(time.perf_counter() - t0)
    times.sort()
    med = times[len(times) // 2]
    print("exec wall times (s):", [round(t, 5) for t in times])
    return int(med * 1e9)


def kernel(x_gcn, x_nn, L_rows, L_cols, L_vals,
           cl1_w, cl1_b, fc1_w, fc1_b, fc2_w, fc2_b, fc3_w, fc3_b,
           enc0_w, enc0_b, enc1_w, enc1_b, enc2_w, enc2_b, enc3_w, enc3_b,
           enc4_w, enc4_b,
           dec0_w, dec1_w, dec2_w, dec3_w, dec4_w, sum_w, sum_b):
    global LAST_EXEC_NS

    f32 = np.float32
    x_gcn = np.asarray(x_gcn, f32)
    x_nn = np.asarray(x_nn, f32)

    # dense L^T in bf16 (duplicates summed)
    Ld = np.zeros((V, V), f32)
    np.add.at(Ld, (np.asarray(L_rows), np.asarray(L_cols)), np.asarray(L_vals, f32))
    LT = np.ascontiguousarray(Ld.T).astype(bf16)

    def c128(a):
        return _chunk128(np.asarray(a, f32)).astype(bf16)

    shared = {"lt": LT, "eye": np.eye(128, dtype=bf16)}
    shared["e0w"] = c128(np.pad(np.asarray(enc0_w, f32).T,
                                ((0, IN_PAD - IN), (0, 0))))
    shared["e1w"] = c128(np.asarray(enc1_w, f32).T)
    shared["e2w"] = c128(np.asarray(enc2_w, f32).T)
    shared["e3w"] = c128(np.asarray(enc3_w, f32).T)
    shared["e4w"] = np.asarray(enc4_w, f32).T.astype(bf16)
    for i, b in enumerate([enc0_b, enc1_b, enc2_b, enc3_b, enc4_b]):
        shared[f"e{i}b"] = np.asarray(b, f32)[None, :].astype(bf16)

    idx = np.arange(3072)
    src = (idx % 1024) * 3 + idx // 1024          # row f*1024+vg <- col vg*3+f
    shared["f1w"] = c128(np.asarray(fc1_w, f32).T[src])
    shared["f2w"] = c128(np.asarray(fc2_w, f32).T)
    shared["f3w"] = c128(np.asarray(fc3_w, f32).T)
    shared["f1b"] = np.asarray(fc1_b, f32)[None, :].astype(bf16)
    shared["f2b"] = np.asarray(fc2_b, f32)[None, :].astype(bf16)
    shared["f3b"] = np.asarray(fc3_b, f32)[None, :].astype(bf16)

    shared["d0"] = np.asarray(dec0_w, f32).astype(bf16)
    shared["d1t"] = np.asarray(dec1_w, f32).T.astype(bf16)
    shared["d2t"] = c128(np.asarray(dec2_w, f32).T)
    shared["d3t"] = c128(np.asarray(dec3_w, f32).T)
    d4t = np.pad(np.asarray(dec4_w, f32).T, ((0, 0), (0, IN_PAD - IN)))
    E = d4t.reshape(4, 128, NT, 512)
    shared["d4b"] = np.ascontiguousarray(
        E.transpose(1, 2, 0, 3).reshape(128, -1)).astype(bf16)
    shared["sw"] = np.asarray(sum_w, f32).T.astype(bf16)
    shared["sb"] = np.asarray(sum_b, f32)[None, :].astype(bf16)

    in_maps = []
    for i in range(NCORES):
        sl = slice(i * BS, (i + 1) * BS)
        xg = x_gcn[sl]                                # [128, 8192]
        xn = np.pad(x_nn[sl], ((0, 0), (0, IN_PAD - IN)))
        m = dict(shared)
        m["x0c"] = _chunk128(xg.T).astype(bf16)
        m["x0t"] = xg.astype(bf16)
        m["xnc"] = _chunk128(xn.T).astype(bf16)
        in_maps.append(m)

    nc = bacc.Bacc("TRN2", target_bir_lowering=False, debug=False)
    nc = _build(nc, np.asarray(cl1_w, f32), np.asarray(cl1_b, f32))

    res = run_bass_kernel_spmd(nc, in_maps, core_ids=list(range(NCORES)))
    LAST_EXEC_NS = res.exec_time_ns

    import os
    if os.environ.get("KERNEL_TIME", "0") == "1":
        try:
            LAST_EXEC_NS = _time_exec(nc, in_maps)
        except Exception as e:
            print("timing failed:", repr(e))

    recon = np.concatenate([r["out_recon"] for r in res.results], axis=0)
    z = np.concatenate([r["out_z"] for r in res.results], axis=0)
    sig = res.results[0]["out_sig"]
    return recon.astype(f32), z.astype(f32), sig.astype(f32)
